# revision 8
# baseline (speedup 1.0000x reference)
"""PointNet Feature Propagation kernel for 8 trn2 NeuronCores.

Sharding: data-parallel over batch B=8 (one batch element per core).
BN (training mode, stats over full batch) uses an AllReduce of per-core
partial sums between the two conv layers.

Self-contained: hardcodes all shapes; host-side work is layout staging only
(transposes, dtype casts, hi/mid/lo bf16 splits, small constant packing).
"""

import sys

sys.path.insert(0, "/opt/trn_rl_repo")

import numpy as np
import ml_dtypes

import concourse.bass as bass
import concourse.mybir as mybir
import concourse.bacc as bacc
import concourse.tile as tile
from concourse.bass_types import AP
from concourse import bass_utils

F32 = mybir.dt.float32
F16 = mybir.dt.float16
BF16 = mybir.dt.bfloat16
U32 = mybir.dt.uint32

B, N, S, D1, D2 = 8, 8192, 2048, 128, 256
C0, C1, C2 = D1 + D2, 256, 128
KNN = 3
KD = 21  # distance-matmul contraction rows (3 coords x 6 split pairs + 3 norm rows)
BN_EPS = 1e-5
P = 128


def _bcast_mid(ap, pos, count):
    """Insert a 0-stride broadcast dim at position `pos` of an AP."""
    dims = [list(d) for d in ap.ap]
    dims.insert(pos, [0, count])
    return AP(ap.tensor, ap.offset, dims)


def build_kernel(tc, t, n_cores, nt):
    """Emit the per-core program. `t` is the dict of dram tensor APs.

    nt = number of 128-query tiles (64 for the full problem).
    """
    nc = tc.nc
    n_loc = nt * P                      # queries on this core
    total = n_cores * n_loc             # BN sample count
    GT = 4                              # query-tiles per gather group
    NCH = n_loc // 512                  # 512-wide column chunks for MLP

    with tc.tile_pool(name="persist", bufs=1) as pp:
        v8 = pp.tile([P, nt * 8], F32)
        ix = pp.tile([P, nt * 8], U32)
        interp = pp.tile([P, nt * 256], F16)
        ft0 = pp.tile([P, n_loc], F16)      # feat^T k-tile 1 (interp ch 0-127)
        ft1 = pp.tile([P, n_loc], F16)      # feat^T k-tile 2 (interp ch 128-255)
        y0r = [pp.tile([P, n_loc], F16, name=f"y0r{i}") for i in range(2)]
        y1r = pp.tile([P, n_loc], F16)
        x1 = [pp.tile([P, n_loc], F16, name=f"x1_{i}") for i in range(2)]
        asb = pp.tile([P, nt], F32)
        w16 = pp.tile([P, nt * KNN], F16)
        w8 = pp.tile([P, nt * KNN * 8], F16)
        ident = pp.tile([P, P], F16)
        bnc0 = pp.tile([P, 6], F32)   # b0,g0,be0 packed [p, {b,g,be} x 2 halves]
        bnc1 = pp.tile([P, 3], F32)
        st0 = [pp.tile([P, NCH * 6], F32, name=f"st0_{i}") for i in range(2)]
        st1 = pp.tile([P, NCH * 6], F32)

        nc.sync.dma_start(asb[:], t["asb"][:])
        nc.sync.dma_start(ident[:], t["ident"][:])
        nc.sync.dma_start(bnc0[:], t["bnc0"][:])
        nc.sync.dma_start(bnc1[:], t["bnc1"][:])

        # ---------------- Phase A: distances + top-3 select ----------------
        with tc.tile_pool(name="dist_in", bufs=1) as dp, \
             tc.tile_pool(name="pse", bufs=2, space="PSUM") as pse:
            dl = dp.tile([KD, n_loc], BF16)
            dr = dp.tile([KD, S], BF16)
            nc.sync.dma_start(dl[:], t["dl"][:])
            nc.sync.dma_start(dr[:], t["dr"][:])

            for i in range(nt):
                e = pse.tile([P, S], F32)
                for j in range(S // 512):
                    nc.tensor.matmul(
                        e[:, j * 512:(j + 1) * 512],
                        dl[:, i * P:(i + 1) * P],
                        dr[:, j * 512:(j + 1) * 512],
                        start=True, stop=True,
                    )
                nc.vector.max(v8[:, i * 8:(i + 1) * 8], e[:])
                nc.vector.max_index(ix[:, i * 8:(i + 1) * 8], v8[:, i * 8:(i + 1) * 8], e[:])

        # ---------------- interpolation weights (batched tiny ops) ---------
        with tc.tile_pool(name="wmath", bufs=1) as wp:
            d3 = wp.tile([P, nt * KNN], F32)
            # d_k = ||xs||^2 - e_k  (e descending => d ascending)
            nc.vector.tensor_tensor(
                out=d3[:].rearrange("p (t k) -> p t k", k=KNN),
                in0=asb[:].to_broadcast([P, nt, KNN]),
                in1=v8[:].rearrange("p (t k) -> p t k", k=8)[:, :, :KNN],
                op=mybir.AluOpType.subtract,
            )
            nc.vector.tensor_scalar_add(d3[:], d3[:], 1e-8)
            r3 = wp.tile([P, nt * KNN], F32)
            nc.vector.reciprocal(r3[:], d3[:])
            rs = wp.tile([P, nt], F32)
            nc.vector.tensor_reduce(
                out=rs[:],
                in_=r3[:].rearrange("p (t k) -> p t k", k=KNN),
                op=mybir.AluOpType.add, axis=mybir.AxisListType.X,
            )
            rsi = wp.tile([P, nt], F32)
            nc.vector.reciprocal(rsi[:], rs[:])
            wf = wp.tile([P, nt * KNN], F32)
            nc.vector.tensor_tensor(
                out=wf[:].rearrange("p (t k) -> p t k", k=KNN),
                in0=r3[:].rearrange("p (t k) -> p t k", k=KNN),
                in1=rsi[:].to_broadcast([P, nt, KNN]),
                op=mybir.AluOpType.mult,
            )
            nc.vector.tensor_copy(w16[:], wf[:])
            nc.vector.tensor_copy(
                w8[:].rearrange("p (g e) -> p g e", e=8),
                w16[:].to_broadcast([P, nt * KNN, 8]),
            )

        # ---------------- Phase B: gather + weighted sum + transpose -------
        # one indirect DMA per (query-tile, k): [128,1] offsets -> [128,256]
        ixv = ix[:].rearrange("p (t k) -> p t k", k=8)
        with tc.tile_pool(name="gb", bufs=2) as gp, \
             tc.tile_pool(name="hb", bufs=2) as hp:
            for g in range(nt // GT):
                gl = GT * KNN * 256
                G = gp.tile([P, gl], F16)
                Gv = G[:].rearrange("p (r c) -> p r c", c=256)
                for r in range(GT * KNN):
                    tq, k = divmod(r, KNN)
                    nc.gpsimd.indirect_dma_start(
                        out=Gv[:, r, :],
                        out_offset=None,
                        in_=t["pnt"][:],
                        in_offset=bass.IndirectOffsetOnAxis(
                            ap=ixv[:, g * GT + tq, k:k + 1], axis=0),
                    )
                H = hp.tile([P, gl], F16)
                w8v = w8[:].rearrange("p (g e) -> p g e", e=8)[
                    :, g * GT * KNN:(g + 1) * GT * KNN, :]
                nc.vector.tensor_mul(
                    H[:].rearrange("p (r ch e) -> p r ch e", ch=32, e=8),
                    G[:].rearrange("p (r ch e) -> p r ch e", ch=32, e=8),
                    _bcast_mid(w8v, 2, 32),
                )
                Hv = H[:].rearrange("p (t k c) -> p t k c", t=GT, k=KNN)
                iv = interp[:].rearrange("p (t c) -> p t c", c=256)[
                    :, g * GT:(g + 1) * GT, :]
                nc.vector.tensor_tensor(out=iv, in0=Hv[:, :, 0], in1=Hv[:, :, 1],
                                        op=mybir.AluOpType.add)
                nc.vector.tensor_tensor(out=iv, in0=iv, in1=Hv[:, :, 2],
                                        op=mybir.AluOpType.add)

        # transpose interp [q, c] -> feat^T tiles [c, q] via identity matmul
        with tc.tile_pool(name="pst", bufs=2, space="PSUM") as pst:
            iv3 = interp[:].rearrange("p (t c) -> p t c", c=256)
            for h, ft in enumerate((ft0, ft1)):
                for q4 in range(nt // 4):
                    ps = pst.tile([P, 512], F32)
                    for j in range(4):
                        qt = q4 * 4 + j
                        nc.tensor.matmul(
                            ps[:, j * P:(j + 1) * P],
                            iv3[:, qt, h * P:(h + 1) * P],
                            ident[:],
                            start=True, stop=True,
                        )
                    nc.scalar.copy(ft[:, q4 * 512:(q4 + 1) * 512], ps[:])

        # ---------------- Phase C: MLP with batch-synced BN ----------------
        with tc.tile_pool(name="mlp_in", bufs=1) as mp, \
             tc.tile_pool(name="psy", bufs=4, space="PSUM") as psy, \
             tc.tile_pool(name="dram", bufs=1, space="DRAM") as dram, \
             tc.tile_pool(name="cc", bufs=1) as ccp:
            psa = mp.tile([P, n_loc], F16)
            nc.sync.dma_start(psa[:], t["psa"][:])
            w0t = mp.tile([P, 6 * P], F16)   # [p, (k 3) (h 2) m] k-tiles x halves
            nc.sync.dma_start(w0t[:], t["w0t"][:])
            w1t = mp.tile([P, 2 * P], F16)
            nc.sync.dma_start(w1t[:], t["w1t"][:])

            ktiles0 = (psa, ft0, ft1)
            for h in range(2):
                for ch in range(NCH):
                    ps = psy.tile([P, 512], F32)
                    for k in range(3):
                        nc.tensor.matmul(
                            ps[:],
                            w0t[:, (k * 2 + h) * P:(k * 2 + h + 1) * P],
                            ktiles0[k][:, ch * 512:(ch + 1) * 512],
                            start=(k == 0), stop=(k == 2),
                        )
                    nc.vector.bn_stats(st0[h][:, ch * 6:(ch + 1) * 6], ps[:])
                    nc.scalar.copy(y0r[h][:, ch * 512:(ch + 1) * 512], ps[:])

            # pack per-core partial sums: [sum_h, sumsq_h] x 2
            ar_in = ccp.tile([P, 4], F32)
            ar_out = ccp.tile([P, 4], F32)
            mv = ccp.tile([P, 4], F32)
            for h in range(2):
                nc.vector.bn_aggr(mv[:, h * 2:(h + 1) * 2], st0[h][:])
            # sum = mean*n_loc ; sumsq = (var + mean^2)*n_loc
            msq = ccp.tile([P, 2], F32)
            mea = mv[:].rearrange("p (h s) -> p h s", s=2)[:, :, 0]
            var = mv[:].rearrange("p (h s) -> p h s", s=2)[:, :, 1]
            nc.vector.tensor_tensor(out=msq[:], in0=mea, in1=mea,
                                    op=mybir.AluOpType.mult)
            nc.vector.tensor_tensor(out=msq[:], in0=msq[:], in1=var,
                                    op=mybir.AluOpType.add)
            arv = ar_in[:].rearrange("p (h s) -> p h s", s=2)
            nc.vector.tensor_scalar_mul(arv[:, :, 0], mea, float(n_loc))
            nc.vector.tensor_scalar_mul(arv[:, :, 1], msq[:], float(n_loc))

            ain_d = dram.tile([P, 4], F32)
            aout_d = dram.tile([P, 4], F32)
            nc.sync.dma_start(ain_d[:], ar_in[:])
            nc.gpsimd.collective_compute(
                "AllReduce", mybir.AluOpType.add,
                replica_groups=[list(range(n_cores))],
                ins=[ain_d.opt()], outs=[aout_d.opt()],
            )
            nc.sync.dma_start(ar_out[:], aout_d[:])

            # global mean/var -> scale/shift ; x1 = relu(y0*scale + shift)
            sc0 = ccp.tile([P, 2], F32)
            sh0 = ccp.tile([P, 2], F32)
            tmp = ccp.tile([P, 2], F32)
            gme = ccp.tile([P, 2], F32)
            aov = ar_out[:].rearrange("p (h s) -> p h s", s=2)
            nc.vector.tensor_scalar_mul(gme[:], aov[:, :, 0], 1.0 / total)
            nc.vector.tensor_scalar_mul(tmp[:], aov[:, :, 1], 1.0 / total)
            # var = E[x^2] - mean^2
            nc.vector.tensor_tensor(out=sc0[:], in0=gme[:], in1=gme[:],
                                    op=mybir.AluOpType.mult)
            nc.vector.tensor_tensor(out=tmp[:], in0=tmp[:], in1=sc0[:],
                                    op=mybir.AluOpType.subtract)
            nc.vector.tensor_scalar_add(tmp[:], tmp[:], BN_EPS)
            nc.scalar.sqrt(tmp[:], tmp[:])
            nc.vector.reciprocal(tmp[:], tmp[:])            # rstd
            b0v = bnc0[:].rearrange("p (s h) -> p s h", s=3)
            nc.vector.tensor_tensor(out=sc0[:], in0=b0v[:, 1, :], in1=tmp[:],
                                    op=mybir.AluOpType.mult)   # g*rstd
            nc.vector.tensor_tensor(out=tmp[:], in0=gme[:], in1=sc0[:],
                                    op=mybir.AluOpType.mult)   # mean*scale
            # conv bias b0 cancels exactly in training-mode BN (y - mean(y))
            nc.vector.tensor_tensor(out=sh0[:], in0=b0v[:, 2, :], in1=tmp[:],
                                    op=mybir.AluOpType.subtract)
            for h in range(2):
                nc.scalar.activation(
                    x1[h][:], y0r[h][:], mybir.ActivationFunctionType.Relu,
                    bias=sh0[:, h:h + 1], scale=sc0[:, h:h + 1],
                )

            # ---- layer 2 ----
            for ch in range(NCH):
                ps = psy.tile([P, 512], F32)
                for k in range(2):
                    nc.tensor.matmul(
                        ps[:],
                        w1t[:, k * P:(k + 1) * P],
                        x1[k][:, ch * 512:(ch + 1) * 512],
                        start=(k == 0), stop=(k == 1),
                    )
                nc.vector.bn_stats(st1[:, ch * 6:(ch + 1) * 6], ps[:])
                nc.scalar.copy(y1r[:, ch * 512:(ch + 1) * 512], ps[:])

            ar_in1 = ccp.tile([P, 2], F32)
            ar_out1 = ccp.tile([P, 2], F32)
            mv1 = ccp.tile([P, 2], F32)
            nc.vector.bn_aggr(mv1[:], st1[:])
            msq1 = ccp.tile([P, 1], F32)
            nc.vector.tensor_tensor(out=msq1[:], in0=mv1[:, 0:1], in1=mv1[:, 0:1],
                                    op=mybir.AluOpType.mult)
            nc.vector.tensor_tensor(out=msq1[:], in0=msq1[:], in1=mv1[:, 1:2],
                                    op=mybir.AluOpType.add)
            nc.vector.tensor_scalar_mul(ar_in1[:, 0:1], mv1[:, 0:1], float(n_loc))
            nc.vector.tensor_scalar_mul(ar_in1[:, 1:2], msq1[:], float(n_loc))
            ain1_d = dram.tile([P, 2], F32)
            aout1_d = dram.tile([P, 2], F32)
            nc.sync.dma_start(ain1_d[:], ar_in1[:])
            nc.gpsimd.collective_compute(
                "AllReduce", mybir.AluOpType.add,
                replica_groups=[list(range(n_cores))],
                ins=[ain1_d.opt()], outs=[aout1_d.opt()],
            )
            nc.sync.dma_start(ar_out1[:], aout1_d[:])

            sc1 = ccp.tile([P, 1], F32)
            sh1 = ccp.tile([P, 1], F32)
            tmp1 = ccp.tile([P, 1], F32)
            gme1 = ccp.tile([P, 1], F32)
            nc.vector.tensor_scalar_mul(gme1[:], ar_out1[:, 0:1], 1.0 / total)
            nc.vector.tensor_scalar_mul(tmp1[:], ar_out1[:, 1:2], 1.0 / total)
            nc.vector.tensor_tensor(out=sc1[:], in0=gme1[:], in1=gme1[:],
                                    op=mybir.AluOpType.mult)
            nc.vector.tensor_tensor(out=tmp1[:], in0=tmp1[:], in1=sc1[:],
                                    op=mybir.AluOpType.subtract)
            nc.vector.tensor_scalar_add(tmp1[:], tmp1[:], BN_EPS)
            nc.scalar.sqrt(tmp1[:], tmp1[:])
            nc.vector.reciprocal(tmp1[:], tmp1[:])
            b1v = bnc1[:].rearrange("p (s h) -> p s h", s=3)
            nc.vector.tensor_tensor(out=sc1[:], in0=b1v[:, 1, :], in1=tmp1[:],
                                    op=mybir.AluOpType.mult)
            nc.vector.tensor_tensor(out=tmp1[:], in0=gme1[:], in1=sc1[:],
                                    op=mybir.AluOpType.mult)
            nc.vector.tensor_tensor(out=sh1[:], in0=b1v[:, 2, :], in1=tmp1[:],
                                    op=mybir.AluOpType.subtract)

            with tc.tile_pool(name="yst", bufs=2) as yp:
                for ch in range(NCH):
                    ys = yp.tile([P, 512], F32)
                    nc.scalar.activation(
                        ys[:], y1r[:, ch * 512:(ch + 1) * 512],
                        mybir.ActivationFunctionType.Relu,
                        bias=sh1[:, 0:1], scale=sc1[:, 0:1],
                    )
                    nc.sync.dma_start(t["y"][:, ch * 512:(ch + 1) * 512], ys[:])


def _declare(nc, nt):
    n_loc = nt * P
    mk = lambda name, shape, dt, kind: nc.dram_tensor(name, shape, dt, kind=kind).ap()
    t = {
        "dl": mk("dl", [KD, n_loc], BF16, "ExternalInput"),
        "dr": mk("dr", [KD, S], BF16, "ExternalInput"),
        "asb": mk("asb", [P, nt], F32, "ExternalInput"),
        "psa": mk("psa", [P, n_loc], F16, "ExternalInput"),
        "pnt": mk("pnt", [S, D2], F16, "ExternalInput"),
        "w0t": mk("w0t", [P, 6 * P], F16, "ExternalInput"),
        "w1t": mk("w1t", [P, 2 * P], F16, "ExternalInput"),
        "bnc0": mk("bnc0", [P, 6], F32, "ExternalInput"),
        "bnc1": mk("bnc1", [P, 3], F32, "ExternalInput"),
        "ident": mk("ident", [P, P], F16, "ExternalInput"),
        "y": mk("y", [P, n_loc], F32, "ExternalOutput"),
    }
    return t


def build_nc(n_cores=8, nt=N // P):
    nc = bacc.Bacc("TRN2", target_bir_lowering=False, debug=False,
                   num_devices=n_cores)
    t = _declare(nc, nt)
    with tile.TileContext(nc) as tc:
        build_kernel(tc, t, n_cores, nt)
    nc.compile()
    return nc


def _split3_bf16(a):
    """3-way bf16 split: a ~= hi + mid + lo (to ~2^-24 rel)."""
    a = a.astype(np.float32)
    hi = a.astype(ml_dtypes.bfloat16)
    r = a - hi.astype(np.float32)
    mid = r.astype(ml_dtypes.bfloat16)
    lo = (r - mid.astype(np.float32)).astype(ml_dtypes.bfloat16)
    return hi, mid, lo


def host_prep_core(xyz_sa_b, xyz_now_b, points_sa_b, points_now_b,
                   W0, b0, g0, be0, W1, b1, g1, be1, nt=N // P):
    """Build the per-core input map (numpy; layout staging only)."""
    n_loc = nt * P
    xs = xyz_sa_b[:, :n_loc].astype(np.float32)        # [3, n]
    xn = xyz_now_b.astype(np.float32)                  # [3, S]

    # distance matmul operands: e = 2*xs.xn - ||xn||^2, via bf16 splits
    a_hi, a_mid, a_lo = _split3_bf16(2.0 * xs)         # [3, n] each
    b_hi, b_mid, b_lo = _split3_bf16(xn)               # [3, S]
    q = -(xn * xn).sum(axis=0, dtype=np.float32)       # [S]
    q_hi, q_mid, q_lo = _split3_bf16(q)
    ones = np.ones((1, n_loc), dtype=ml_dtypes.bfloat16)
    zs = np.zeros((1, S), dtype=ml_dtypes.bfloat16)

    dl_rows, dr_rows = [], []
    for c in range(3):
        # pairs: (hi,hi) (hi,mid) (mid,hi) (hi,lo) (lo,hi) (mid,mid)
        for (al, bl) in (
            (a_hi, b_hi), (a_hi, b_mid), (a_mid, b_hi),
            (a_hi, b_lo), (a_lo, b_hi), (a_mid, b_mid),
        ):
            dl_rows.append(al[c:c + 1])
            dr_rows.append(bl[c:c + 1])
    for qq in (q_hi, q_mid, q_lo):
        dl_rows.append(ones)
        dr_rows.append(qq[None, :].astype(ml_dtypes.bfloat16))
    dl = np.concatenate(dl_rows, axis=0)               # [21, n]
    dr = np.concatenate(dr_rows, axis=0)               # [21, S]

    A = (xs * xs).sum(axis=0, dtype=np.float32)        # [n]
    asb = A.reshape(nt, P).T.copy()                    # [p, t]

    psa = points_sa_b[:, :n_loc].astype(np.float16)    # [128, n]
    pnt = points_now_b.T.copy().astype(np.float16)     # [S, 256]

    # W0^T packed [128, (k h) 128]: lhsT slice for k-tile k, out-half h
    w0t = np.zeros((P, 6 * P), dtype=np.float16)
    W0T = W0.T.astype(np.float16)                      # [384, 256]
    for k in range(3):
        for h in range(2):
            w0t[:, (k * 2 + h) * P:(k * 2 + h + 1) * P] = \
                W0T[k * P:(k + 1) * P, h * P:(h + 1) * P]
    w1t = np.zeros((P, 2 * P), dtype=np.float16)
    W1T = W1.T.astype(np.float16)                      # [256, 128]
    for k in range(2):
        w1t[:, k * P:(k + 1) * P] = W1T[k * P:(k + 1) * P, :]

    bnc0 = np.stack([b0.reshape(2, P), g0.reshape(2, P), be0.reshape(2, P)],
                    axis=0).astype(np.float32)          # [3, 2, 128]
    bnc0 = bnc0.transpose(2, 0, 1).reshape(P, 6).copy()  # [p, (s h)]
    bnc1 = np.stack([b1, g1, be1], axis=0).astype(np.float32)  # [3, 128]
    bnc1 = bnc1.T.copy()                                # [p, 3]

    ident = np.eye(P, dtype=np.float16)

    return {
        "dl": np.ascontiguousarray(dl),
        "dr": np.ascontiguousarray(dr),
        "asb": np.ascontiguousarray(asb),
        "psa": np.ascontiguousarray(psa),
        "pnt": pnt,
        "w0t": w0t,
        "w1t": w1t,
        "bnc0": bnc0,
        "bnc1": bnc1,
        "ident": ident,
    }


_NC_CACHE = {}


def kernel(xyz_sa, xyz_now, points_sa, points_now,
           W0, b0, g0, be0, W1, b1, g1, be1, k,
           _trace=False, _trace_kwargs=None):
    assert int(k) == KNN
    key = (8, N // P)
    if key not in _NC_CACHE:
        _NC_CACHE[key] = build_nc(*key)
    nc = _NC_CACHE[key]

    xyz_sa = np.asarray(xyz_sa); xyz_now = np.asarray(xyz_now)
    points_sa = np.asarray(points_sa); points_now = np.asarray(points_now)
    in_maps = [
        host_prep_core(xyz_sa[b], xyz_now[b], points_sa[b], points_now[b],
                       W0, b0, g0, be0, W1, b1, g1, be1)
        for b in range(B)
    ]
    res = bass_utils.run_bass_kernel_spmd(
        nc, in_maps, core_ids=list(range(8)),
        trace=_trace, **(_trace_kwargs or {}),
    )
    out = np.stack([res.results[b]["y"] for b in range(B)], axis=0)
    if _trace:
        return out.astype(np.float32), res
    return out.astype(np.float32)


# revision 9
# speedup vs baseline: 1.2038x; 1.2038x over previous
"""PointNet Feature Propagation kernel for 8 trn2 NeuronCores.

Sharding: data-parallel over batch B=8 (one batch element per core).
BN (training mode, stats over full batch) uses an AllReduce of per-core
partial sums between the two conv layers.

Self-contained: hardcodes all shapes; host-side work is layout staging only
(transposes, dtype casts, hi/mid/lo bf16 splits, small constant packing).
"""

import sys

sys.path.insert(0, "/opt/trn_rl_repo")

import numpy as np
import ml_dtypes

import concourse.bass as bass
import concourse.mybir as mybir
import concourse.bacc as bacc
import concourse.tile as tile
from concourse.bass_types import AP
from concourse import bass_utils

F32 = mybir.dt.float32
F16 = mybir.dt.float16
BF16 = mybir.dt.bfloat16
U32 = mybir.dt.uint32

B, N, S, D1, D2 = 8, 8192, 2048, 128, 256
C0, C1, C2 = D1 + D2, 256, 128
KNN = 3
KD = 21  # distance-matmul contraction rows (3 coords x 6 split pairs + 3 norm rows)
BN_EPS = 1e-5
P = 128


def _bcast_mid(ap, pos, count):
    """Insert a 0-stride broadcast dim at position `pos` of an AP."""
    dims = [list(d) for d in ap.ap]
    dims.insert(pos, [0, count])
    return AP(ap.tensor, ap.offset, dims)


def build_kernel(tc, t, n_cores, nt):
    """Emit the per-core program. `t` is the dict of dram tensor APs.

    nt = number of 128-query tiles (64 for the full problem).
    """
    nc = tc.nc
    n_loc = nt * P                      # queries on this core
    total = n_cores * n_loc             # BN sample count
    GT = 4                              # query-tiles per gather group
    NCH = n_loc // 512                  # 512-wide column chunks for MLP

    with tc.tile_pool(name="persist", bufs=1) as pp:
        v8 = pp.tile([P, nt * 8], F32)
        ix = pp.tile([P, nt * 8], U32)
        interp = pp.tile([P, nt * 256], F16)
        ft0 = pp.tile([P, n_loc], F16)      # feat^T k-tile 1 (interp ch 0-127)
        ft1 = pp.tile([P, n_loc], F16)      # feat^T k-tile 2 (interp ch 128-255)
        y0r = [pp.tile([P, n_loc], F16, name=f"y0r{i}") for i in range(2)]
        y1r = pp.tile([P, n_loc], F16)
        x1 = [pp.tile([P, n_loc], F16, name=f"x1_{i}") for i in range(2)]
        asb = pp.tile([P, nt], F32)
        w16 = pp.tile([P, nt * KNN], F16)
        w8 = pp.tile([P, nt * KNN * 8], F16)
        ident = pp.tile([P, P], F16)
        bnc0 = pp.tile([P, 6], F32)   # b0,g0,be0 packed [p, {b,g,be} x 2 halves]
        bnc1 = pp.tile([P, 3], F32)
        st0 = [pp.tile([P, NCH * 6], F32, name=f"st0_{i}") for i in range(2)]
        st1 = pp.tile([P, NCH * 6], F32)

        nc.sync.dma_start(asb[:], t["asb"][:])
        nc.sync.dma_start(ident[:], t["ident"][:])
        nc.sync.dma_start(bnc0[:], t["bnc0"][:])
        nc.sync.dma_start(bnc1[:], t["bnc1"][:])

        # ------- Phase A+B fused: distances, top-3, weights, gather, interp --
        # Per GT-tile group: matmul+Max8+MaxIndex (DVE), weight math (tiny),
        # indirect gathers (GPSIMD/DMA — overlap the next group's scan), then
        # the weighted-sum.  Gathers hide under the DVE scan this way.
        ixv = ix[:].rearrange("p (t k) -> p t k", k=8)
        GK = GT * KNN
        with tc.tile_pool(name="dist_in", bufs=1) as dp, \
             tc.tile_pool(name="pse", bufs=2, space="PSUM") as pse, \
             tc.tile_pool(name="wmath", bufs=2) as wp, \
             tc.tile_pool(name="gb", bufs=3) as gp, \
             tc.tile_pool(name="hb", bufs=2) as hp:
            dl = dp.tile([KD, n_loc], BF16)
            dr = dp.tile([KD, S], BF16)
            nc.sync.dma_start(dl[:], t["dl"][:])
            nc.sync.dma_start(dr[:], t["dr"][:])

            for g in range(nt // GT):
                for i in range(g * GT, (g + 1) * GT):
                    e = pse.tile([P, S], F32, tag="e")
                    for j in range(S // 512):
                        nc.tensor.matmul(
                            e[:, j * 512:(j + 1) * 512],
                            dl[:, i * P:(i + 1) * P],
                            dr[:, j * 512:(j + 1) * 512],
                            start=True, stop=True,
                        )
                    nc.vector.max(v8[:, i * 8:(i + 1) * 8], e[:])
                    nc.vector.max_index(
                        ix[:, i * 8:(i + 1) * 8], v8[:, i * 8:(i + 1) * 8], e[:])

                # weights for this group: d_k = ||xs||^2 - e_k
                sl3 = slice(g * GK, (g + 1) * GK)
                d3 = wp.tile([P, GK], F32, tag="d3")
                nc.vector.tensor_tensor(
                    out=d3[:].rearrange("p (t k) -> p t k", k=KNN),
                    in0=asb[:, g * GT:(g + 1) * GT].to_broadcast([P, GT, KNN]),
                    in1=v8[:].rearrange("p (t k) -> p t k", k=8)[
                        :, g * GT:(g + 1) * GT, :KNN],
                    op=mybir.AluOpType.subtract,
                )
                nc.vector.tensor_scalar_add(d3[:], d3[:], 1e-8)
                r3 = wp.tile([P, GK], F32, tag="r3")
                nc.vector.reciprocal(r3[:], d3[:])
                rs = wp.tile([P, GT], F32, tag="rs")
                nc.vector.tensor_reduce(
                    out=rs[:], in_=r3[:].rearrange("p (t k) -> p t k", k=KNN),
                    op=mybir.AluOpType.add, axis=mybir.AxisListType.X,
                )
                rsi = wp.tile([P, GT], F32, tag="rsi")
                nc.vector.reciprocal(rsi[:], rs[:])
                wf = wp.tile([P, GK], F32, tag="wf")
                nc.vector.tensor_tensor(
                    out=wf[:].rearrange("p (t k) -> p t k", k=KNN),
                    in0=r3[:].rearrange("p (t k) -> p t k", k=KNN),
                    in1=rsi[:].to_broadcast([P, GT, KNN]),
                    op=mybir.AluOpType.mult,
                )
                nc.vector.tensor_copy(w16[:, sl3], wf[:])
                nc.vector.tensor_copy(
                    w8[:].rearrange("p (g e) -> p g e", e=8)[:, sl3, :],
                    w16[:, sl3].to_broadcast([P, GK, 8]),
                )

                # gathers for this group (GPSIMD queue; overlaps next scans)
                gl = GK * 256
                G = gp.tile([P, gl], F16, tag="G")
                Gv = G[:].rearrange("p (r c) -> p r c", c=256)
                for r in range(GK):
                    tq, k = divmod(r, KNN)
                    nc.gpsimd.indirect_dma_start(
                        out=Gv[:, r, :],
                        out_offset=None,
                        in_=t["pnt"][:],
                        in_offset=bass.IndirectOffsetOnAxis(
                            ap=ixv[:, g * GT + tq, k:k + 1], axis=0),
                    )
                H = hp.tile([P, gl], F16, tag="H")
                w8v = w8[:].rearrange("p (g e) -> p g e", e=8)[:, sl3, :]
                nc.vector.tensor_mul(
                    H[:].rearrange("p (r ch e) -> p r ch e", ch=32, e=8),
                    G[:].rearrange("p (r ch e) -> p r ch e", ch=32, e=8),
                    _bcast_mid(w8v, 2, 32),
                )
                Hv = H[:].rearrange("p (t k c) -> p t k c", t=GT, k=KNN)
                iv = interp[:].rearrange("p (t c) -> p t c", c=256)[
                    :, g * GT:(g + 1) * GT, :]
                nc.vector.tensor_tensor(out=iv, in0=Hv[:, :, 0], in1=Hv[:, :, 1],
                                        op=mybir.AluOpType.add)
                nc.vector.tensor_tensor(out=iv, in0=iv, in1=Hv[:, :, 2],
                                        op=mybir.AluOpType.add)

        # transpose interp [q, c] -> feat^T tiles [c, q] via identity matmul
        with tc.tile_pool(name="pst", bufs=2, space="PSUM") as pst:
            iv3 = interp[:].rearrange("p (t c) -> p t c", c=256)
            for h, ft in enumerate((ft0, ft1)):
                for q4 in range(nt // 4):
                    ps = pst.tile([P, 512], F32)
                    for j in range(4):
                        qt = q4 * 4 + j
                        nc.tensor.matmul(
                            ps[:, j * P:(j + 1) * P],
                            iv3[:, qt, h * P:(h + 1) * P],
                            ident[:],
                            start=True, stop=True,
                        )
                    nc.scalar.copy(ft[:, q4 * 512:(q4 + 1) * 512], ps[:])

        # ---------------- Phase C: MLP with batch-synced BN ----------------
        with tc.tile_pool(name="mlp_in", bufs=1) as mp, \
             tc.tile_pool(name="psy", bufs=4, space="PSUM") as psy, \
             tc.tile_pool(name="dram", bufs=1, space="DRAM") as dram, \
             tc.tile_pool(name="cc", bufs=1) as ccp:
            psa = mp.tile([P, n_loc], F16)
            nc.sync.dma_start(psa[:], t["psa"][:])
            w0t = mp.tile([P, 6 * P], F16)   # [p, (k 3) (h 2) m] k-tiles x halves
            nc.sync.dma_start(w0t[:], t["w0t"][:])
            w1t = mp.tile([P, 2 * P], F16)
            nc.sync.dma_start(w1t[:], t["w1t"][:])

            ktiles0 = (psa, ft0, ft1)
            for h in range(2):
                for ch in range(NCH):
                    ps = psy.tile([P, 512], F32)
                    for k in range(3):
                        nc.tensor.matmul(
                            ps[:],
                            w0t[:, (k * 2 + h) * P:(k * 2 + h + 1) * P],
                            ktiles0[k][:, ch * 512:(ch + 1) * 512],
                            start=(k == 0), stop=(k == 2),
                        )
                    nc.vector.bn_stats(st0[h][:, ch * 6:(ch + 1) * 6], ps[:])
                    nc.scalar.copy(y0r[h][:, ch * 512:(ch + 1) * 512], ps[:])

            # pack per-core partial sums: [sum_h, sumsq_h] x 2
            ar_in = ccp.tile([P, 4], F32)
            ar_out = ccp.tile([P, 4], F32)
            mv = ccp.tile([P, 4], F32)
            for h in range(2):
                nc.vector.bn_aggr(mv[:, h * 2:(h + 1) * 2], st0[h][:])
            # sum = mean*n_loc ; sumsq = (var + mean^2)*n_loc
            msq = ccp.tile([P, 2], F32)
            mea = mv[:].rearrange("p (h s) -> p h s", s=2)[:, :, 0]
            var = mv[:].rearrange("p (h s) -> p h s", s=2)[:, :, 1]
            nc.vector.tensor_tensor(out=msq[:], in0=mea, in1=mea,
                                    op=mybir.AluOpType.mult)
            nc.vector.tensor_tensor(out=msq[:], in0=msq[:], in1=var,
                                    op=mybir.AluOpType.add)
            arv = ar_in[:].rearrange("p (h s) -> p h s", s=2)
            nc.vector.tensor_scalar_mul(arv[:, :, 0], mea, float(n_loc))
            nc.vector.tensor_scalar_mul(arv[:, :, 1], msq[:], float(n_loc))

            ain_d = dram.tile([P, 4], F32)
            aout_d = dram.tile([P, 4], F32)
            nc.sync.dma_start(ain_d[:], ar_in[:])
            nc.gpsimd.collective_compute(
                "AllReduce", mybir.AluOpType.add,
                replica_groups=[list(range(n_cores))],
                ins=[ain_d.opt()], outs=[aout_d.opt()],
            )
            nc.sync.dma_start(ar_out[:], aout_d[:])

            # global mean/var -> scale/shift ; x1 = relu(y0*scale + shift)
            sc0 = ccp.tile([P, 2], F32)
            sh0 = ccp.tile([P, 2], F32)
            tmp = ccp.tile([P, 2], F32)
            gme = ccp.tile([P, 2], F32)
            aov = ar_out[:].rearrange("p (h s) -> p h s", s=2)
            nc.vector.tensor_scalar_mul(gme[:], aov[:, :, 0], 1.0 / total)
            nc.vector.tensor_scalar_mul(tmp[:], aov[:, :, 1], 1.0 / total)
            # var = E[x^2] - mean^2
            nc.vector.tensor_tensor(out=sc0[:], in0=gme[:], in1=gme[:],
                                    op=mybir.AluOpType.mult)
            nc.vector.tensor_tensor(out=tmp[:], in0=tmp[:], in1=sc0[:],
                                    op=mybir.AluOpType.subtract)
            nc.vector.tensor_scalar_add(tmp[:], tmp[:], BN_EPS)
            nc.scalar.sqrt(tmp[:], tmp[:])
            nc.vector.reciprocal(tmp[:], tmp[:])            # rstd
            b0v = bnc0[:].rearrange("p (s h) -> p s h", s=3)
            nc.vector.tensor_tensor(out=sc0[:], in0=b0v[:, 1, :], in1=tmp[:],
                                    op=mybir.AluOpType.mult)   # g*rstd
            nc.vector.tensor_tensor(out=tmp[:], in0=gme[:], in1=sc0[:],
                                    op=mybir.AluOpType.mult)   # mean*scale
            # conv bias b0 cancels exactly in training-mode BN (y - mean(y))
            nc.vector.tensor_tensor(out=sh0[:], in0=b0v[:, 2, :], in1=tmp[:],
                                    op=mybir.AluOpType.subtract)
            for h in range(2):
                nc.scalar.activation(
                    x1[h][:], y0r[h][:], mybir.ActivationFunctionType.Relu,
                    bias=sh0[:, h:h + 1], scale=sc0[:, h:h + 1],
                )

            # ---- layer 2 ----
            for ch in range(NCH):
                ps = psy.tile([P, 512], F32)
                for k in range(2):
                    nc.tensor.matmul(
                        ps[:],
                        w1t[:, k * P:(k + 1) * P],
                        x1[k][:, ch * 512:(ch + 1) * 512],
                        start=(k == 0), stop=(k == 1),
                    )
                nc.vector.bn_stats(st1[:, ch * 6:(ch + 1) * 6], ps[:])
                nc.scalar.copy(y1r[:, ch * 512:(ch + 1) * 512], ps[:])

            ar_in1 = ccp.tile([P, 2], F32)
            ar_out1 = ccp.tile([P, 2], F32)
            mv1 = ccp.tile([P, 2], F32)
            nc.vector.bn_aggr(mv1[:], st1[:])
            msq1 = ccp.tile([P, 1], F32)
            nc.vector.tensor_tensor(out=msq1[:], in0=mv1[:, 0:1], in1=mv1[:, 0:1],
                                    op=mybir.AluOpType.mult)
            nc.vector.tensor_tensor(out=msq1[:], in0=msq1[:], in1=mv1[:, 1:2],
                                    op=mybir.AluOpType.add)
            nc.vector.tensor_scalar_mul(ar_in1[:, 0:1], mv1[:, 0:1], float(n_loc))
            nc.vector.tensor_scalar_mul(ar_in1[:, 1:2], msq1[:], float(n_loc))
            ain1_d = dram.tile([P, 2], F32)
            aout1_d = dram.tile([P, 2], F32)
            nc.sync.dma_start(ain1_d[:], ar_in1[:])
            nc.gpsimd.collective_compute(
                "AllReduce", mybir.AluOpType.add,
                replica_groups=[list(range(n_cores))],
                ins=[ain1_d.opt()], outs=[aout1_d.opt()],
            )
            nc.sync.dma_start(ar_out1[:], aout1_d[:])

            sc1 = ccp.tile([P, 1], F32)
            sh1 = ccp.tile([P, 1], F32)
            tmp1 = ccp.tile([P, 1], F32)
            gme1 = ccp.tile([P, 1], F32)
            nc.vector.tensor_scalar_mul(gme1[:], ar_out1[:, 0:1], 1.0 / total)
            nc.vector.tensor_scalar_mul(tmp1[:], ar_out1[:, 1:2], 1.0 / total)
            nc.vector.tensor_tensor(out=sc1[:], in0=gme1[:], in1=gme1[:],
                                    op=mybir.AluOpType.mult)
            nc.vector.tensor_tensor(out=tmp1[:], in0=tmp1[:], in1=sc1[:],
                                    op=mybir.AluOpType.subtract)
            nc.vector.tensor_scalar_add(tmp1[:], tmp1[:], BN_EPS)
            nc.scalar.sqrt(tmp1[:], tmp1[:])
            nc.vector.reciprocal(tmp1[:], tmp1[:])
            b1v = bnc1[:].rearrange("p (s h) -> p s h", s=3)
            nc.vector.tensor_tensor(out=sc1[:], in0=b1v[:, 1, :], in1=tmp1[:],
                                    op=mybir.AluOpType.mult)
            nc.vector.tensor_tensor(out=tmp1[:], in0=gme1[:], in1=sc1[:],
                                    op=mybir.AluOpType.mult)
            nc.vector.tensor_tensor(out=sh1[:], in0=b1v[:, 2, :], in1=tmp1[:],
                                    op=mybir.AluOpType.subtract)

            with tc.tile_pool(name="yst", bufs=2) as yp:
                for ch in range(NCH):
                    ys = yp.tile([P, 512], F32)
                    nc.scalar.activation(
                        ys[:], y1r[:, ch * 512:(ch + 1) * 512],
                        mybir.ActivationFunctionType.Relu,
                        bias=sh1[:, 0:1], scale=sc1[:, 0:1],
                    )
                    nc.sync.dma_start(t["y"][:, ch * 512:(ch + 1) * 512], ys[:])


def _declare(nc, nt):
    n_loc = nt * P
    mk = lambda name, shape, dt, kind: nc.dram_tensor(name, shape, dt, kind=kind).ap()
    t = {
        "dl": mk("dl", [KD, n_loc], BF16, "ExternalInput"),
        "dr": mk("dr", [KD, S], BF16, "ExternalInput"),
        "asb": mk("asb", [P, nt], F32, "ExternalInput"),
        "psa": mk("psa", [P, n_loc], F16, "ExternalInput"),
        "pnt": mk("pnt", [S, D2], F16, "ExternalInput"),
        "w0t": mk("w0t", [P, 6 * P], F16, "ExternalInput"),
        "w1t": mk("w1t", [P, 2 * P], F16, "ExternalInput"),
        "bnc0": mk("bnc0", [P, 6], F32, "ExternalInput"),
        "bnc1": mk("bnc1", [P, 3], F32, "ExternalInput"),
        "ident": mk("ident", [P, P], F16, "ExternalInput"),
        "y": mk("y", [P, n_loc], F32, "ExternalOutput"),
    }
    return t


def build_nc(n_cores=8, nt=N // P):
    nc = bacc.Bacc("TRN2", target_bir_lowering=False, debug=False,
                   num_devices=n_cores)
    t = _declare(nc, nt)
    with tile.TileContext(nc) as tc:
        build_kernel(tc, t, n_cores, nt)
    nc.compile()
    return nc


def _split3_bf16(a):
    """3-way bf16 split: a ~= hi + mid + lo (to ~2^-24 rel)."""
    a = a.astype(np.float32)
    hi = a.astype(ml_dtypes.bfloat16)
    r = a - hi.astype(np.float32)
    mid = r.astype(ml_dtypes.bfloat16)
    lo = (r - mid.astype(np.float32)).astype(ml_dtypes.bfloat16)
    return hi, mid, lo


def host_prep_core(xyz_sa_b, xyz_now_b, points_sa_b, points_now_b,
                   W0, b0, g0, be0, W1, b1, g1, be1, nt=N // P):
    """Build the per-core input map (numpy; layout staging only)."""
    n_loc = nt * P
    xs = xyz_sa_b[:, :n_loc].astype(np.float32)        # [3, n]
    xn = xyz_now_b.astype(np.float32)                  # [3, S]

    # distance matmul operands: e = 2*xs.xn - ||xn||^2, via bf16 splits
    a_hi, a_mid, a_lo = _split3_bf16(2.0 * xs)         # [3, n] each
    b_hi, b_mid, b_lo = _split3_bf16(xn)               # [3, S]
    q = -(xn * xn).sum(axis=0, dtype=np.float32)       # [S]
    q_hi, q_mid, q_lo = _split3_bf16(q)
    ones = np.ones((1, n_loc), dtype=ml_dtypes.bfloat16)
    zs = np.zeros((1, S), dtype=ml_dtypes.bfloat16)

    dl_rows, dr_rows = [], []
    for c in range(3):
        # pairs: (hi,hi) (hi,mid) (mid,hi) (hi,lo) (lo,hi) (mid,mid)
        for (al, bl) in (
            (a_hi, b_hi), (a_hi, b_mid), (a_mid, b_hi),
            (a_hi, b_lo), (a_lo, b_hi), (a_mid, b_mid),
        ):
            dl_rows.append(al[c:c + 1])
            dr_rows.append(bl[c:c + 1])
    for qq in (q_hi, q_mid, q_lo):
        dl_rows.append(ones)
        dr_rows.append(qq[None, :].astype(ml_dtypes.bfloat16))
    dl = np.concatenate(dl_rows, axis=0)               # [21, n]
    dr = np.concatenate(dr_rows, axis=0)               # [21, S]

    A = (xs * xs).sum(axis=0, dtype=np.float32)        # [n]
    asb = A.reshape(nt, P).T.copy()                    # [p, t]

    psa = points_sa_b[:, :n_loc].astype(np.float16)    # [128, n]
    pnt = points_now_b.T.copy().astype(np.float16)     # [S, 256]

    # W0^T packed [128, (k h) 128]: lhsT slice for k-tile k, out-half h
    w0t = np.zeros((P, 6 * P), dtype=np.float16)
    W0T = W0.T.astype(np.float16)                      # [384, 256]
    for k in range(3):
        for h in range(2):
            w0t[:, (k * 2 + h) * P:(k * 2 + h + 1) * P] = \
                W0T[k * P:(k + 1) * P, h * P:(h + 1) * P]
    w1t = np.zeros((P, 2 * P), dtype=np.float16)
    W1T = W1.T.astype(np.float16)                      # [256, 128]
    for k in range(2):
        w1t[:, k * P:(k + 1) * P] = W1T[k * P:(k + 1) * P, :]

    bnc0 = np.stack([b0.reshape(2, P), g0.reshape(2, P), be0.reshape(2, P)],
                    axis=0).astype(np.float32)          # [3, 2, 128]
    bnc0 = bnc0.transpose(2, 0, 1).reshape(P, 6).copy()  # [p, (s h)]
    bnc1 = np.stack([b1, g1, be1], axis=0).astype(np.float32)  # [3, 128]
    bnc1 = bnc1.T.copy()                                # [p, 3]

    ident = np.eye(P, dtype=np.float16)

    return {
        "dl": np.ascontiguousarray(dl),
        "dr": np.ascontiguousarray(dr),
        "asb": np.ascontiguousarray(asb),
        "psa": np.ascontiguousarray(psa),
        "pnt": pnt,
        "w0t": w0t,
        "w1t": w1t,
        "bnc0": bnc0,
        "bnc1": bnc1,
        "ident": ident,
    }


_NC_CACHE = {}


def kernel(xyz_sa, xyz_now, points_sa, points_now,
           W0, b0, g0, be0, W1, b1, g1, be1, k,
           _trace=False, _trace_kwargs=None):
    assert int(k) == KNN
    key = (8, N // P)
    if key not in _NC_CACHE:
        _NC_CACHE[key] = build_nc(*key)
    nc = _NC_CACHE[key]

    xyz_sa = np.asarray(xyz_sa); xyz_now = np.asarray(xyz_now)
    points_sa = np.asarray(points_sa); points_now = np.asarray(points_now)
    in_maps = [
        host_prep_core(xyz_sa[b], xyz_now[b], points_sa[b], points_now[b],
                       W0, b0, g0, be0, W1, b1, g1, be1)
        for b in range(B)
    ]
    res = bass_utils.run_bass_kernel_spmd(
        nc, in_maps, core_ids=list(range(8)),
        trace=_trace, **(_trace_kwargs or {}),
    )
    out = np.stack([res.results[b]["y"] for b in range(B)], axis=0)
    if _trace:
        return out.astype(np.float32), res
    return out.astype(np.float32)
